# revision 16
# baseline (speedup 1.0000x reference)
"""Trainium2 Bass kernel: low-rank (LoRA-style) linear with 2:4 soft-threshold
pruned weights, fp16 matmul / fp32 accumulate.

  wA = soft_threshold24(weight_A) * scale_A          # [IN, R]
  wB = soft_threshold24(weight_B) * scale_B          # [OUT, R]
  x_proj = f16(x) @ f16(wA)            (f32 accum)   # [N, R]
  out    = f16(x_proj) @ f16(wB).T + bias            # [N, OUT]

Sharding: data-parallel over the token dim across 8 cores (2048 tokens/core),
small weights replicated. No collectives.

Per-core pipeline (8 groups of 256 tokens):
  sync-DMA x f32 -> GPSIMD cast f16 -> PE transpose (f16, via identity) ->
  ACT copy PSUM->SBUF -> 32 accumulating f16 matmuls vs wA (256-wide moving)
  -> DVE cast to f16 + ones row -> f16 matmuls vs wB.T (bias row folded in)
  -> DVE copy PSUM->SBUF f32 -> ACT-ring DMA store.
"""

import sys

import numpy as np

if "/opt/trn_rl_repo" not in sys.path:
    sys.path.insert(0, "/opt/trn_rl_repo")

B, S, IN_F, OUT_F, RANK = 4, 4096, 4096, 4096, 64
N_CORES = 8
N_TOK = B * S                   # 16384
T_CORE = N_TOK // N_CORES       # 2048 tokens per core
P = 128
TT = 2                          # token tiles per group
GTOK = TT * P                   # 256 tokens per group
N_GRP = T_CORE // GTOK          # 8 groups per core
N_IB = IN_F // P                # 32 input-feature blocks
MM2_N = 512
N_OB = OUT_F // MM2_N           # 8 output column groups

_CACHE = {}


def _soft_threshold_weights(nc, pool, w_dram, scale, out_f16):
    """Emit IR computing out_f16 = f16(soft_threshold24(w_dram) * scale).

    w_dram: [IN_or_OUT, RANK] f32, viewed as [P, blocks, RANK] with
    partition = row-within-block. out_f16: [P, blocks, RANK] f16 tile.
    Loads via gpsimd (SWDGE) to keep the HWDGE rings free for x traffic.
    """
    import concourse.mybir as mybir

    f32 = mybir.dt.float32
    nb = w_dram.shape[0] // P
    wf = pool.tile([P, nb, RANK], f32, tag="wstage", name="wstage")
    nc.scalar.dma_start(wf[:], w_dram[:].rearrange("(b p) r -> p b r", p=P))

    g = wf[:].rearrange("p b (g q) -> p b g q", q=4)
    gj = [g[:, :, :, j : j + 1] for j in range(4)]
    ash = [P, nb, RANK // 4, 1]
    amin = mybir.AluOpType.min
    amx = mybir.AluOpType.max
    mul = mybir.AluOpType.mult

    # |a_j| = max(-a_j, a_j)
    wneg = pool.tile([P, nb, RANK], f32, tag="wneg")
    nc.vector.tensor_scalar_mul(wneg[:], wf[:], -1.0)
    ng = wneg[:].rearrange("p b (g q) -> p b g q", q=4)
    ab = [pool.tile(ash, f32, tag=f"abs{j}", name=f"abs{j}") for j in range(4)]
    for j in range(4):
        nc.vector.tensor_tensor(ab[j][:], gj[j], ng[:, :, :, j : j + 1], op=amx)
    m1 = pool.tile(ash, f32, tag="m1")
    M1 = pool.tile(ash, f32, tag="M1")
    m2 = pool.tile(ash, f32, tag="abs0")
    M2 = pool.tile(ash, f32, tag="abs1")
    nc.vector.tensor_tensor(m1[:], ab[0][:], ab[1][:], op=amin)
    nc.vector.tensor_tensor(M1[:], ab[0][:], ab[1][:], op=amx)
    nc.vector.tensor_tensor(m2[:], ab[2][:], ab[3][:], op=amin)
    nc.vector.tensor_tensor(M2[:], ab[2][:], ab[3][:], op=amx)
    # 2nd smallest of the 4 = min(max(m1, m2), min(M1, M2))
    t = pool.tile(ash, f32, tag="abs2")
    nc.vector.tensor_tensor(m1[:], m1[:], m2[:], op=amx)
    nc.vector.tensor_tensor(M1[:], M1[:], M2[:], op=amin)
    nc.vector.tensor_tensor(t[:], m1[:], M1[:], op=amin)

    # t4: threshold broadcast back over the group-of-4 axis
    t4 = pool.tile([P, nb, RANK], f32, tag="t4")
    h4 = t4[:].rearrange("p b (g q) -> p b g q", q=4)
    for j in range(4):
        nc.vector.tensor_copy(h4[:, :, :, j : j + 1], t[:])
    # s = w - clip(w, -t, t)  (3 contiguous ops)
    thr = pool.tile([P, nb, RANK], f32, tag="wthr", name="wthr")
    nt4 = pool.tile([P, nb, RANK], f32, tag="wneg", name="nt4")
    nc.vector.tensor_scalar_mul(nt4[:], t4[:], -1.0)
    nc.vector.tensor_tensor(thr[:], wf[:], t4[:], op=amin)
    nc.vector.tensor_tensor(thr[:], thr[:], nt4[:], op=amx)
    nc.vector.tensor_sub(thr[:], wf[:], thr[:])
    if scale != 1.0:
        nc.vector.tensor_scalar_mul(thr[:], thr[:], float(scale))
    ck = nb // 4
    for c in range(4):
        nc.vector.tensor_copy(out_f16[:, c * ck : (c + 1) * ck, :],
                              thr[:, c * ck : (c + 1) * ck, :])


def _build(scale_a, scale_b):
    import concourse.mybir as mybir
    import concourse.tile as tile
    from concourse import bacc
    from concourse.bass import ts
    from concourse.masks import make_identity

    f32, f16 = mybir.dt.float32, mybir.dt.float16

    nc = bacc.Bacc("TRN2", target_bir_lowering=False, debug=False,
                   enable_asserts=False)
    x_d = nc.dram_tensor("x", [T_CORE, IN_F], f32, kind="ExternalInput")
    wa_d = nc.dram_tensor("weight_A", [IN_F, RANK], f32, kind="ExternalInput")
    wb_d = nc.dram_tensor("weight_B", [OUT_F, RANK], f32, kind="ExternalInput")
    b_d = nc.dram_tensor("bias", [1, OUT_F], f32, kind="ExternalInput")
    o_d = nc.dram_tensor("out", [T_CORE, OUT_F], f32, kind="ExternalOutput")

    with tile.TileContext(nc) as tc:
        with (
            tc.tile_pool(name="const", bufs=1) as constp,
            tc.tile_pool(name="wtmp", bufs=1) as wtmp,
            tc.tile_pool(name="xin32", bufs=3) as xin32,
            tc.tile_pool(name="xin16", bufs=3) as xin16,
            tc.tile_pool(name="xtp", bufs=2) as xtp,
            tc.tile_pool(name="outp", bufs=2) as outp,
            tc.tile_pool(name="proj", bufs=4) as projp,
            tc.tile_pool(name="pst", bufs=3, space="PSUM") as pst,
            tc.tile_pool(name="ps1", bufs=2, space="PSUM") as ps1p,
            tc.tile_pool(name="ps2", bufs=3, space="PSUM") as ps2p,
        ):
            ident16 = constp.tile([P, P], f16)
            make_identity(nc, ident16[:])

            # --- weight A first (mm1 of group 0 only needs wa16) ---
            wa16 = constp.tile([P, N_IB, RANK], f16)
            _soft_threshold_weights(nc, wtmp, wa_d, scale_a, wa16)

            # --- weight B: threshold, cast, transpose on PE ---
            wbt = constp.tile([RANK + 1, OUT_F], f16)  # wB.T (+ bias row)
            wb16 = wtmp.tile([P, N_IB, RANK], f16, tag="wb16")
            _soft_threshold_weights(nc, wtmp, wb_d, scale_b, wb16)
            # psum slots from the ps2 pool: mm2 waits for wbt anyway, so no
            # false slot-rotation dependency on the main-loop transposes.
            for b in range(OUT_F // P):
                pw = ps2p.tile([P, MM2_N], f16, tag="ps2", name="pw")
                nc.tensor.transpose(pw[:RANK, :P], wb16[:, b, :], ident16[:])
                nc.scalar.copy(wbt[0:RANK, ts(b, P)], pw[:RANK, :P])
            # bias row (row RANK): f32 -> f16
            bstage = xin32.tile([1, OUT_F], f32, tag="xt32", name="bstage")
            nc.scalar.dma_start(bstage[:], b_d[:])
            nc.vector.tensor_copy(wbt[RANK : RANK + 1, :], bstage[:])



            # --- main loop: 8 groups of 256 tokens ---
            for g in range(N_GRP):
                xt16 = []
                for tt in range(TT):
                    i = g * TT + tt
                    xt32 = xin32.tile([P, IN_F], f32, name="xt32")
                    nc.sync.dma_start(xt32[:], x_d[ts(i, P), :])
                    x16 = xin16.tile([P, IN_F], f16, name="x16")
                    nc.scalar.copy(x16[:], xt32[:])
                    xt16.append(x16)

                # transpose [t, in] -> [in, t] on PE; pack 8 transposes
                # (4 blocks x 2 token-tiles) per PSUM bank; ACT copies out
                xT = xtp.tile([P, N_IB, GTOK], f16)
                for q in range(N_IB // 4):
                    pt = pst.tile([P, 4 * GTOK], f16, tag="ptx", name="pt")
                    for bb in range(4):
                        b = 4 * q + bb
                        for tt in range(TT):
                            nc.tensor.transpose(
                                pt[:, bb * GTOK + tt * P : bb * GTOK + (tt + 1) * P],
                                xt16[tt][:, ts(b, P)], ident16[:])
                    dst = xT[:, ts(q, 4), :].rearrange("p a b -> p (a b)")
                    nc.scalar.copy(dst, pt[:])

                # mm1: x_projT[r, t] = sum_i wa[i, r] * xT[i, t], 256-wide
                ps1 = ps1p.tile([RANK, GTOK], f32)
                for b in range(N_IB):
                    nc.tensor.matmul(ps1[:], wa16[:, b, :], xT[:, b, :],
                                     start=(b == 0), stop=(b == N_IB - 1))

                xpa = projp.tile([RANK + 1, GTOK], f16)
                nc.any.tensor_copy(out=xpa[0:RANK, :], in_=ps1[:])
                nc.vector.memset(xpa[RANK : RANK + 1, :], 1.0)

                # mm2 per token tile: out[t, o] = x_projT.T @ wbt (+ bias row)
                for tt in range(TT):
                    i = g * TT + tt
                    ob = outp.tile([P, OUT_F], f32, name="ob", tag="ob")
                    for j in range(N_OB):
                        ps2 = ps2p.tile([P, MM2_N], f32, tag="ps2", name="ps2")
                        nc.tensor.matmul(ps2[:], xpa[:, ts(tt, P)],
                                         wbt[:, ts(j, MM2_N)],
                                         start=True, stop=True)
                        nc.any.tensor_copy(out=ob[:, ts(j, MM2_N)], in_=ps2[:])
                    nc.scalar.dma_start(o_d[ts(i, P), :], ob[:])

    nc.compile()
    return nc


def get_nc(scale_a, scale_b):
    key = (float(scale_a), float(scale_b))
    if key not in _CACHE:
        _CACHE[key] = _build(*key)
    return _CACHE[key]


def kernel(x, weight_A, weight_B, bias, scale_A, scale_B):
    from concourse.bass_utils import run_bass_kernel_spmd

    x = np.ascontiguousarray(np.asarray(x, dtype=np.float32))
    wa = np.ascontiguousarray(np.asarray(weight_A, dtype=np.float32))
    wb = np.ascontiguousarray(np.asarray(weight_B, dtype=np.float32))
    bi = np.ascontiguousarray(np.asarray(bias, dtype=np.float32)).reshape(1, OUT_F)
    sa = float(np.asarray(scale_A))
    sb = float(np.asarray(scale_B))

    nc = get_nc(sa, sb)

    xf = x.reshape(N_TOK, IN_F)
    in_maps = [
        {
            "x": xf[c * T_CORE : (c + 1) * T_CORE],
            "weight_A": wa,
            "weight_B": wb,
            "bias": bi,
        }
        for c in range(N_CORES)
    ]
    res = run_bass_kernel_spmd(nc, in_maps, core_ids=list(range(N_CORES)))
    out = np.concatenate([r["out"] for r in res.results], axis=0)
    return out.reshape(B, S, OUT_F)


# revision 17
# speedup vs baseline: 1.0118x; 1.0118x over previous
"""Trainium2 Bass kernel: low-rank (LoRA-style) linear with 2:4 soft-threshold
pruned weights, fp16 matmul / fp32 accumulate.

  wA = soft_threshold24(weight_A) * scale_A          # [IN, R]
  wB = soft_threshold24(weight_B) * scale_B          # [OUT, R]
  x_proj = f16(x) @ f16(wA)            (f32 accum)   # [N, R]
  out    = f16(x_proj) @ f16(wB).T + bias            # [N, OUT]

Sharding: data-parallel over the token dim across 8 cores (2048 tokens/core),
small weights replicated. No collectives.

Per-core pipeline (8 groups of 256 tokens):
  sync-DMA x f32 -> GPSIMD cast f16 -> PE transpose (f16, via identity) ->
  ACT copy PSUM->SBUF -> 32 accumulating f16 matmuls vs wA (256-wide moving)
  -> DVE cast to f16 + ones row -> f16 matmuls vs wB.T (bias row folded in)
  -> DVE copy PSUM->SBUF f32 -> ACT-ring DMA store.
"""

import sys

import numpy as np

if "/opt/trn_rl_repo" not in sys.path:
    sys.path.insert(0, "/opt/trn_rl_repo")

B, S, IN_F, OUT_F, RANK = 4, 4096, 4096, 4096, 64
N_CORES = 8
N_TOK = B * S                   # 16384
T_CORE = N_TOK // N_CORES       # 2048 tokens per core
P = 128
TT = 2                          # token tiles per group
GTOK = TT * P                   # 256 tokens per group
N_GRP = T_CORE // GTOK          # 8 groups per core
N_IB = IN_F // P                # 32 input-feature blocks
MM2_N = 512
N_OB = OUT_F // MM2_N           # 8 output column groups

_CACHE = {}


def _soft_threshold_weights(nc, pool, w_dram, scale, out_f16):
    """Emit IR computing out_f16 = f16(soft_threshold24(w_dram) * scale).

    w_dram: [IN_or_OUT, RANK] f32, viewed as [P, blocks, RANK] with
    partition = row-within-block. out_f16: [P, blocks, RANK] f16 tile.
    Loads via gpsimd (SWDGE) to keep the HWDGE rings free for x traffic.
    """
    import concourse.mybir as mybir

    f32 = mybir.dt.float32
    nb = w_dram.shape[0] // P
    wf = pool.tile([P, nb, RANK], f32, tag="wstage", name="wstage")
    nc.scalar.dma_start(wf[:], w_dram[:].rearrange("(b p) r -> p b r", p=P))

    g = wf[:].rearrange("p b (g q) -> p b g q", q=4)
    gj = [g[:, :, :, j : j + 1] for j in range(4)]
    ash = [P, nb, RANK // 4, 1]
    amin = mybir.AluOpType.min
    amx = mybir.AluOpType.max
    mul = mybir.AluOpType.mult

    # |a_j| = max(-a_j, a_j)
    wneg = pool.tile([P, nb, RANK], f32, tag="wneg")
    nc.vector.tensor_scalar_mul(wneg[:], wf[:], -1.0)
    ng = wneg[:].rearrange("p b (g q) -> p b g q", q=4)
    ab = [pool.tile(ash, f32, tag=f"abs{j}", name=f"abs{j}") for j in range(4)]
    for j in range(4):
        nc.vector.tensor_tensor(ab[j][:], gj[j], ng[:, :, :, j : j + 1], op=amx)
    m1 = pool.tile(ash, f32, tag="m1")
    M1 = pool.tile(ash, f32, tag="M1")
    m2 = pool.tile(ash, f32, tag="abs0")
    M2 = pool.tile(ash, f32, tag="abs1")
    nc.vector.tensor_tensor(m1[:], ab[0][:], ab[1][:], op=amin)
    nc.vector.tensor_tensor(M1[:], ab[0][:], ab[1][:], op=amx)
    nc.vector.tensor_tensor(m2[:], ab[2][:], ab[3][:], op=amin)
    nc.vector.tensor_tensor(M2[:], ab[2][:], ab[3][:], op=amx)
    # 2nd smallest of the 4 = min(max(m1, m2), min(M1, M2))
    t = pool.tile(ash, f32, tag="abs2")
    nc.vector.tensor_tensor(m1[:], m1[:], m2[:], op=amx)
    nc.vector.tensor_tensor(M1[:], M1[:], M2[:], op=amin)
    nc.vector.tensor_tensor(t[:], m1[:], M1[:], op=amin)

    # t4: threshold broadcast back over the group-of-4 axis
    t4 = pool.tile([P, nb, RANK], f32, tag="t4")
    h4 = t4[:].rearrange("p b (g q) -> p b g q", q=4)
    for j in range(4):
        nc.vector.tensor_copy(h4[:, :, :, j : j + 1], t[:])
    # s = w - clip(w, -t, t)  (3 contiguous ops)
    thr = pool.tile([P, nb, RANK], f32, tag="wthr", name="wthr")
    nt4 = pool.tile([P, nb, RANK], f32, tag="wneg", name="nt4")
    nc.vector.tensor_scalar_mul(nt4[:], t4[:], -1.0)
    nc.vector.tensor_tensor(thr[:], wf[:], t4[:], op=amin)
    nc.vector.tensor_tensor(thr[:], thr[:], nt4[:], op=amx)
    nc.vector.tensor_sub(thr[:], wf[:], thr[:])
    if scale != 1.0:
        nc.vector.tensor_scalar_mul(thr[:], thr[:], float(scale))
    if out_f16 is not None:
        ck = nb // 4
        for c in range(4):
            nc.vector.tensor_copy(out_f16[:, c * ck : (c + 1) * ck, :],
                                  thr[:, c * ck : (c + 1) * ck, :])
    return thr


def _build(scale_a, scale_b):
    import concourse.mybir as mybir
    import concourse.tile as tile
    from concourse import bacc
    from concourse.bass import ts
    from concourse.masks import make_identity

    f32, f16 = mybir.dt.float32, mybir.dt.float16

    nc = bacc.Bacc("TRN2", target_bir_lowering=False, debug=False,
                   enable_asserts=False)
    x_d = nc.dram_tensor("x", [T_CORE, IN_F], f32, kind="ExternalInput")
    wa_d = nc.dram_tensor("weight_A", [IN_F, RANK], f32, kind="ExternalInput")
    wb_d = nc.dram_tensor("weight_B", [OUT_F, RANK], f32, kind="ExternalInput")
    b_d = nc.dram_tensor("bias", [1, OUT_F], f32, kind="ExternalInput")
    o_d = nc.dram_tensor("out", [T_CORE, OUT_F], f32, kind="ExternalOutput")

    with tile.TileContext(nc) as tc:
        with (
            tc.tile_pool(name="const", bufs=1) as constp,
            tc.tile_pool(name="wtmp", bufs=1) as wtmp,
            tc.tile_pool(name="xin32", bufs=3) as xin32,
            tc.tile_pool(name="xtp", bufs=3) as xtp,
            tc.tile_pool(name="outp", bufs=2) as outp,
            tc.tile_pool(name="proj", bufs=4) as projp,
            tc.tile_pool(name="pst", bufs=3, space="PSUM") as pst,
            tc.tile_pool(name="ps1", bufs=2, space="PSUM") as ps1p,
            tc.tile_pool(name="ps2", bufs=3, space="PSUM") as ps2p,
        ):
            ident32 = constp.tile([P, P], f32)
            make_identity(nc, ident32[:])

            # --- weight A first (mm1 of group 0 only needs wa16) ---
            wa16 = constp.tile([P, N_IB, RANK], f16)
            _soft_threshold_weights(nc, wtmp, wa_d, scale_a, wa16)

            # --- weight B: threshold, cast, transpose on PE ---
            wbt = constp.tile([RANK + 1, OUT_F], f16)  # wB.T (+ bias row)
            thr_b = _soft_threshold_weights(nc, wtmp, wb_d, scale_b, None)
            # psum slots from the ps2 pool: mm2 waits for wbt anyway, so no
            # false slot-rotation dependency on the main-loop transposes.
            for b in range(OUT_F // P):
                pw = ps2p.tile([P, MM2_N], f32, tag="ps2", name="pw")
                nc.tensor.transpose(pw[:RANK, :P], thr_b[:, b, :], ident32[:])
                nc.scalar.copy(wbt[0:RANK, ts(b, P)], pw[:RANK, :P])
            # bias row (row RANK): f32 -> f16
            bstage = xin32.tile([1, OUT_F], f32, tag="xt32", name="bstage")
            nc.scalar.dma_start(bstage[:], b_d[:])
            nc.vector.tensor_copy(wbt[RANK : RANK + 1, :], bstage[:])



            # --- main loop: 8 groups of 256 tokens ---
            for g in range(N_GRP):
                xts = []
                for tt in range(TT):
                    i = g * TT + tt
                    xt32 = xin32.tile([P, IN_F], f32, name="xt32")
                    nc.sync.dma_start(xt32[:], x_d[ts(i, P), :])
                    xts.append(xt32)

                # transpose [t, in] -> [in, t] on PE in f32; ACT copy casts
                # PSUM f32 -> SBUF f16 (4 transposes per PSUM bank)
                xT = xtp.tile([P, N_IB, GTOK], f16)
                for q in range(N_IB // 4):
                    for tt in range(TT):
                        pt = pst.tile([P, 4 * P], f32, tag="ptx", name="pt")
                        for bb in range(4):
                            b = 4 * q + bb
                            nc.tensor.transpose(pt[:, ts(bb, P)],
                                                xts[tt][:, ts(b, P)], ident32[:])
                        dst = xT[:, 4 * q : 4 * q + 4, ts(tt, P)]
                        nc.scalar.copy(dst, pt[:].rearrange("p (a b) -> p a b", a=4))

                # mm1: x_projT[r, t] = sum_i wa[i, r] * xT[i, t], 256-wide
                ps1 = ps1p.tile([RANK, GTOK], f32)
                for b in range(N_IB):
                    nc.tensor.matmul(ps1[:], wa16[:, b, :], xT[:, b, :],
                                     start=(b == 0), stop=(b == N_IB - 1))

                xpa = projp.tile([RANK + 1, GTOK], f16)
                nc.any.tensor_copy(out=xpa[0:RANK, :], in_=ps1[:])
                nc.vector.memset(xpa[RANK : RANK + 1, :], 1.0)

                # mm2 per token tile: out[t, o] = x_projT.T @ wbt (+ bias row)
                for tt in range(TT):
                    i = g * TT + tt
                    ob = outp.tile([P, OUT_F], f32, name="ob", tag="ob")
                    for j in range(N_OB):
                        ps2 = ps2p.tile([P, MM2_N], f32, tag="ps2", name="ps2")
                        nc.tensor.matmul(ps2[:], xpa[:, ts(tt, P)],
                                         wbt[:, ts(j, MM2_N)],
                                         start=True, stop=True)
                        nc.any.tensor_copy(out=ob[:, ts(j, MM2_N)], in_=ps2[:])
                    nc.scalar.dma_start(o_d[ts(i, P), :], ob[:])

    nc.compile()
    return nc


def get_nc(scale_a, scale_b):
    key = (float(scale_a), float(scale_b))
    if key not in _CACHE:
        _CACHE[key] = _build(*key)
    return _CACHE[key]


def kernel(x, weight_A, weight_B, bias, scale_A, scale_B):
    from concourse.bass_utils import run_bass_kernel_spmd

    x = np.ascontiguousarray(np.asarray(x, dtype=np.float32))
    wa = np.ascontiguousarray(np.asarray(weight_A, dtype=np.float32))
    wb = np.ascontiguousarray(np.asarray(weight_B, dtype=np.float32))
    bi = np.ascontiguousarray(np.asarray(bias, dtype=np.float32)).reshape(1, OUT_F)
    sa = float(np.asarray(scale_A))
    sb = float(np.asarray(scale_B))

    nc = get_nc(sa, sb)

    xf = x.reshape(N_TOK, IN_F)
    in_maps = [
        {
            "x": xf[c * T_CORE : (c + 1) * T_CORE],
            "weight_A": wa,
            "weight_B": wb,
            "bias": bi,
        }
        for c in range(N_CORES)
    ]
    res = run_bass_kernel_spmd(nc, in_maps, core_ids=list(range(N_CORES)))
    out = np.concatenate([r["out"] for r in res.results], axis=0)
    return out.reshape(B, S, OUT_F)
